# revision 7
# baseline (speedup 1.0000x reference)
"""Differentiable nearest-neighbor search (vq_codebook) on 8 TRN2 NeuronCores.

reference computes, per row i of feats0:
    dists[i, j] = ||x_i||^2 - 2 x_i.y_j + ||y_j||^2
    probs = softmax(-dists / max(temp^2, 1e-4))
    idx = argmax(probs);  asgn = one_hot(idx)
    asgn_diff = asgn - stop_grad(probs) + probs     (forward value == asgn exactly)

The forward value is an exact one-hot (hot entries exactly 1.0, all else 0.0),
and idx = argmax_j (x_i.y_j - 0.5||y_j||^2) in f32.

Strategy (8 cores, data-parallel over B*N0 rows, 2048 rows/core):
  host prep: sorts the codebook by ||y||^2 so that each 32-wide window of
    columns has a near-constant bias -0.5||y||^2 (midpoint bbar_w, radius
    delta_w).  Ships x, y(sorted) as bf16 plus the [1, 256] bias row.
  device: bf16 matmuls score all candidates (f32 PSUM), DVE reduces each
    PSUM tile to 32-wide window maxima, adds the per-window bias row, and
    max/max_index pick the top window W1 and the top-2 biased window maxima
    u1, u2 per row.  The 512MB zero output is written by DMA.
  host finish: exactly rescores (f64) the 32 columns of W1 plus the columns
    of the 8 highest-spread (tail) windows; if the best exact candidate
    beats u2 by more than the coarse-score error bound, the winner is the
    true argmax (all other windows' members are provably below it);
    otherwise the row falls back to an exact full-row argmax.  The host
    writes the 16384 ones into the device-zeroed output.
"""

import numpy as np

N_CORES = 8
B, N, D = 2, 8192, 128
ROWS_PER_CORE = B * N // N_CORES          # 2048
RT_PER_CORE = ROWS_PER_CORE // 128        # 16 row-tiles of 128 rows
W = 32                                    # window width for the device argmax
NWIN = N // W                             # 256 windows per row
T_EXT = 8                                 # tail windows always rescored on host
EB_MM = 0.30                              # bf16 matmul coarse error bound (measured max 0.181)

TRACE = False          # set by test.py to capture a neuron-profile
LAST_RESULTS = None    # BassKernelResults of the last run (for test.py)

_COMPILED = {}


def _build():
    import concourse.bacc as bacc
    import concourse.tile as tile
    import concourse.mybir as mybir
    from contextlib import ExitStack

    dt = mybir.dt
    nc = bacc.Bacc("TRN2", target_bir_lowering=False, debug=False,
                   num_devices=N_CORES)

    xb_ap = nc.dram_tensor("xb", [128, ROWS_PER_CORE], dt.bfloat16,
                           kind="ExternalInput").ap()
    yb_ap = nc.dram_tensor("yb", [128, N], dt.bfloat16,
                           kind="ExternalInput").ap()
    bb_ap = nc.dram_tensor("bb", [1, NWIN], dt.float32,
                           kind="ExternalInput").ap()
    asgn_ap = nc.dram_tensor("asgn", [ROWS_PER_CORE, N], dt.float32,
                             kind="ExternalOutput").ap()
    w1_ap = nc.dram_tensor("w1", [128, RT_PER_CORE], dt.int32,
                           kind="ExternalOutput").ap()
    u1_ap = nc.dram_tensor("u1", [128, RT_PER_CORE], dt.float32,
                           kind="ExternalOutput").ap()
    u2_ap = nc.dram_tensor("u2", [128, RT_PER_CORE], dt.float32,
                           kind="ExternalOutput").ap()

    with tile.TileContext(nc) as tc, ExitStack() as ctx:
        const = ctx.enter_context(tc.tile_pool(name="const", bufs=1))
        small = ctx.enter_context(tc.tile_pool(name="small", bufs=2))
        psum = ctx.enter_context(tc.tile_pool(name="psum", bufs=2, space="PSUM"))

        xb = const.tile([128, ROWS_PER_CORE], dt.bfloat16)
        nc.sync.dma_start(xb[:], xb_ap[:])
        yb = const.tile([128, N], dt.bfloat16)
        nc.sync.dma_start(yb[:], yb_ap[:])
        bb = const.tile([1, NWIN], dt.float32)
        nc.sync.dma_start(bb[:], bb_ap[:])
        bb128 = const.tile([128, NWIN], dt.float32)
        nc.gpsimd.partition_broadcast(bb128[:], bb[:])

        zero_tile = const.tile([128, N], dt.float32)
        nc.gpsimd.memset(zero_tile[:], 0.0)

        w1_all = const.tile([128, RT_PER_CORE], dt.int32)
        u1_all = const.tile([128, RT_PER_CORE], dt.float32)
        u2_all = const.tile([128, RT_PER_CORE], dt.float32)

        for rt in range(RT_PER_CORE):
            lhs = xb[:, rt * 128:(rt + 1) * 128]
            smax = small.tile([128, NWIN], dt.float32, tag="smax")
            for g in range(4):
                ps = psum.tile([128, 2048], dt.float32, tag="ps")
                for q in range(4):
                    jt = g * 4 + q
                    nc.tensor.matmul(ps[:, q * 512:(q + 1) * 512], lhs,
                                     yb[:, jt * 512:(jt + 1) * 512],
                                     start=True, stop=True)
                nc.vector.tensor_reduce(
                    smax[:, g * 64:(g + 1) * 64],
                    ps[:].rearrange("p (g w) -> p g w", w=W),
                    axis=mybir.AxisListType.X, op=mybir.AluOpType.max)
            # add the per-window bias row (broadcast across partitions)
            nc.vector.tensor_tensor(smax[:], smax[:], bb128[:],
                                    op=mybir.AluOpType.add)

            m8 = small.tile([128, 8], dt.float32, tag="m8")
            wi8 = small.tile([128, 8], dt.uint32, tag="wi8")
            nc.vector.max(m8[:], smax[:])
            nc.vector.max_index(wi8[:], m8[:], smax[:])

            nc.scalar.copy(u1_all[:, rt:rt + 1], m8[:, 0:1])
            nc.scalar.copy(u2_all[:, rt:rt + 1], m8[:, 1:2])
            nc.gpsimd.tensor_copy(w1_all[:, rt:rt + 1], wi8[:, 0:1])

            # zero this row-tile's 4MB slab of the output
            nc.sync.dma_start(asgn_ap[rt * 128:(rt + 1) * 128, :], zero_tile[:])

        nc.sync.dma_start(w1_ap[:], w1_all[:])
        nc.sync.dma_start(u1_ap[:], u1_all[:])
        nc.sync.dma_start(u2_ap[:], u2_all[:])

    nc.compile()
    return nc


def _get_nc():
    if "nc" not in _COMPILED:
        _COMPILED["nc"] = _build()
    return _COMPILED["nc"]


def kernel(feats0, feats1, temp):
    global LAST_RESULTS
    import ml_dtypes
    from concourse import bass_utils

    feats0 = np.asarray(feats0, dtype=np.float32)
    feats1 = np.asarray(feats1, dtype=np.float32)

    nc = _get_nc()

    # --- host prep: sort codebook, window bias metadata ----------------
    perm_b, ybT_b, bb_b, dcap_b, ext_b, y2_b = [], [], [], [], [], []
    for b in range(B):
        y = feats1[b]
        y2 = (y.astype(np.float64) ** 2).sum(-1)
        perm = np.argsort(y2, kind="stable")
        y2s = y2[perm]
        y2w = y2s.reshape(NWIN, W)
        bbar = (-0.25 * (y2w.min(-1) + y2w.max(-1))).astype(np.float32)
        delta = 0.25 * (y2w.max(-1) - y2w.min(-1))
        ext = np.argsort(-delta)[:T_EXT]
        dcap = float(np.delete(delta, ext).max())
        perm_b.append(perm)
        ybT_b.append(np.ascontiguousarray(
            y[perm].T.astype(ml_dtypes.bfloat16)))
        bb_b.append(bbar.reshape(1, NWIN))
        dcap_b.append(dcap)
        ext_b.append(ext)
        y2_b.append(y2)

    in_maps = []
    for c in range(N_CORES):
        b, r = divmod(c, N_CORES // B)
        rows = slice(r * ROWS_PER_CORE, (r + 1) * ROWS_PER_CORE)
        in_maps.append({
            "xb": np.ascontiguousarray(
                feats0[b, rows, :].T.astype(ml_dtypes.bfloat16)),
            "yb": ybT_b[b],
            "bb": bb_b[b],
        })

    res = bass_utils.run_bass_kernel_spmd(
        nc, in_maps, core_ids=list(range(N_CORES)), trace=TRACE)
    LAST_RESULTS = res

    # --- host finish ---------------------------------------------------
    asgn = np.empty((B, N, N), dtype=np.float32)
    idx = np.empty((B, N), dtype=np.int32)
    cores_per_b = N_CORES // B
    rows_all = np.arange(N)
    for b in range(B):
        w1_rows = np.empty(N, dtype=np.int64)
        u1_rows = np.empty(N, dtype=np.float32)
        u2_rows = np.empty(N, dtype=np.float32)
        for r in range(cores_per_b):
            c = b * cores_per_b + r
            o = res.results[c]
            rows = slice(r * ROWS_PER_CORE, (r + 1) * ROWS_PER_CORE)
            asgn[b, rows, :] = o["asgn"]
            # device tile layout [partition p, row-tile rt] -> row rt*128+p
            w1_rows[rows] = o["w1"].T.reshape(-1)
            u1_rows[rows] = o["u1"].T.reshape(-1)
            u2_rows[rows] = o["u2"].T.reshape(-1)

        x = feats0[b]
        y = feats1[b]
        perm = perm_b[b]
        ext = ext_b[b]
        y2 = y2_b[b]
        x64 = x.astype(np.float64)
        big = np.int64(1 << 40)

        # exact scores of the tail (extreme-spread) windows — shared columns
        ext_cols = perm[(ext[:, None] * W +
                         np.arange(W)[None, :]).reshape(-1)]      # [T*W]
        e_ext = x64 @ y[ext_cols].T.astype(np.float64) \
            - 0.5 * y2[ext_cols][None, :]                         # [N, T*W]
        e_ext_best = e_ext.max(-1)
        idx_ext = np.where(e_ext >= e_ext_best[:, None],
                           ext_cols[None, :], big).min(-1)

        # exact scores of each row's winning window, grouped by window id
        e_w1_best = np.empty(N)
        idx_w1 = np.empty(N, dtype=np.int64)
        for wv in np.unique(w1_rows):
            rows = np.nonzero(w1_rows == wv)[0]
            wcols = perm[wv * W + np.arange(W)]                   # [W]
            e = x64[rows] @ y[wcols].T.astype(np.float64) \
                - 0.5 * y2[wcols][None, :]                        # [nr, W]
            eb = e.max(-1)
            e_w1_best[rows] = eb
            idx_w1[rows] = np.where(e >= eb[:, None],
                                    wcols[None, :], big).min(-1)

        # combine (smallest original column wins exact ties)
        e_best = np.maximum(e_w1_best, e_ext_best)
        idx_b = np.where(
            e_w1_best > e_ext_best, idx_w1,
            np.where(e_ext_best > e_w1_best, idx_ext,
                     np.minimum(idx_w1, idx_ext)))

        # margin test; fall back to exact full-row argmax inside the margin
        fb = (e_best - u2_rows) <= (EB_MM + dcap_b[b])
        if fb.any():
            s = x64[fb] @ y.T.astype(np.float64) - 0.5 * y2[None, :]
            idx_b[fb] = s.argmax(-1)

        idx[b] = idx_b.astype(np.int32)
        asgn[b, rows_all, idx_b] = 1.0

    return asgn, idx


# revision 8
# speedup vs baseline: 1.0049x; 1.0049x over previous
"""Differentiable nearest-neighbor search (vq_codebook) on 8 TRN2 NeuronCores.

reference computes, per row i of feats0:
    dists[i, j] = ||x_i||^2 - 2 x_i.y_j + ||y_j||^2
    probs = softmax(-dists / max(temp^2, 1e-4))
    idx = argmax(probs);  asgn = one_hot(idx)
    asgn_diff = asgn - stop_grad(probs) + probs     (forward value == asgn exactly)

The forward value is an exact one-hot (hot entries exactly 1.0, all else 0.0),
and idx = argmax_j (x_i.y_j - 0.5||y_j||^2) in f32.

Strategy (8 cores, data-parallel over B*N0 rows, 2048 rows/core):
  host prep: sorts the codebook by ||y||^2 so that each 32-wide window of
    columns has a near-constant bias -0.5||y||^2 (midpoint bbar_w, radius
    delta_w).  Ships x, y(sorted) as bf16 plus the [1, 256] bias row.
  device: bf16 matmuls score all candidates (f32 PSUM), DVE reduces each
    PSUM tile to 32-wide window maxima, adds the per-window bias row, and
    max/max_index pick the top window W1 and the top-2 biased window maxima
    u1, u2 per row.  The 512MB zero output is written by DMA.
  host finish: exactly rescores (f64) the 32 columns of W1 plus the columns
    of the 8 highest-spread (tail) windows; if the best exact candidate
    beats u2 by more than the coarse-score error bound, the winner is the
    true argmax (all other windows' members are provably below it);
    otherwise the row falls back to an exact full-row argmax.  The host
    writes the 16384 ones into the device-zeroed output.
"""

import numpy as np

N_CORES = 8
B, N, D = 2, 8192, 128
ROWS_PER_CORE = B * N // N_CORES          # 2048
RT_PER_CORE = ROWS_PER_CORE // 128        # 16 row-tiles of 128 rows
W = 32                                    # window width for the device argmax
NWIN = N // W                             # 256 windows per row
T_EXT = 8                                 # tail windows always rescored on host
EB_MM = 0.30                              # bf16 matmul coarse error bound (measured max 0.181)

TRACE = False          # set by test.py to capture a neuron-profile
LAST_RESULTS = None    # BassKernelResults of the last run (for test.py)

_COMPILED = {}


def _build():
    import concourse.bacc as bacc
    import concourse.tile as tile
    import concourse.mybir as mybir
    from contextlib import ExitStack

    dt = mybir.dt
    nc = bacc.Bacc("TRN2", target_bir_lowering=False, debug=False,
                   num_devices=N_CORES)

    xb_ap = nc.dram_tensor("xb", [128, ROWS_PER_CORE], dt.bfloat16,
                           kind="ExternalInput").ap()
    yb_ap = nc.dram_tensor("yb", [128, N], dt.bfloat16,
                           kind="ExternalInput").ap()
    bb_ap = nc.dram_tensor("bb", [1, NWIN], dt.float32,
                           kind="ExternalInput").ap()
    asgn_ap = nc.dram_tensor("asgn", [ROWS_PER_CORE, N], dt.float32,
                             kind="ExternalOutput").ap()
    w1_ap = nc.dram_tensor("w1", [128, RT_PER_CORE], dt.int32,
                           kind="ExternalOutput").ap()
    u1_ap = nc.dram_tensor("u1", [128, RT_PER_CORE], dt.float32,
                           kind="ExternalOutput").ap()
    u2_ap = nc.dram_tensor("u2", [128, RT_PER_CORE], dt.float32,
                           kind="ExternalOutput").ap()

    with tile.TileContext(nc) as tc, ExitStack() as ctx:
        const = ctx.enter_context(tc.tile_pool(name="const", bufs=1))
        small = ctx.enter_context(tc.tile_pool(name="small", bufs=2))
        psum = ctx.enter_context(tc.tile_pool(name="psum", bufs=2, space="PSUM"))

        xb = const.tile([128, ROWS_PER_CORE], dt.bfloat16)
        nc.scalar.dma_start(xb[:], xb_ap[:])
        yb = const.tile([128, N], dt.bfloat16)
        nc.scalar.dma_start(yb[:], yb_ap[:])
        bb = const.tile([1, NWIN], dt.float32)
        nc.scalar.dma_start(bb[:], bb_ap[:])
        bb128 = const.tile([128, NWIN], dt.float32)
        nc.gpsimd.partition_broadcast(bb128[:], bb[:])

        zero_tile = const.tile([128, N], dt.float32)
        nc.vector.memset(zero_tile[:], 0.0)

        w1_all = const.tile([128, RT_PER_CORE], dt.int32)
        u1_all = const.tile([128, RT_PER_CORE], dt.float32)
        u2_all = const.tile([128, RT_PER_CORE], dt.float32)

        for rt in range(RT_PER_CORE):
            lhs = xb[:, rt * 128:(rt + 1) * 128]
            smax = small.tile([128, NWIN], dt.float32, tag="smax")
            for g in range(4):
                ps = psum.tile([128, 2048], dt.float32, tag="ps")
                for q in range(4):
                    jt = g * 4 + q
                    nc.tensor.matmul(ps[:, q * 512:(q + 1) * 512], lhs,
                                     yb[:, jt * 512:(jt + 1) * 512],
                                     start=True, stop=True)
                nc.vector.tensor_reduce(
                    smax[:, g * 64:(g + 1) * 64],
                    ps[:].rearrange("p (g w) -> p g w", w=W),
                    axis=mybir.AxisListType.X, op=mybir.AluOpType.max)
            # add the per-window bias row (broadcast across partitions)
            nc.vector.tensor_tensor(smax[:], smax[:], bb128[:],
                                    op=mybir.AluOpType.add)

            m8 = small.tile([128, 8], dt.float32, tag="m8")
            wi8 = small.tile([128, 8], dt.uint32, tag="wi8")
            nc.vector.max(m8[:], smax[:])
            nc.vector.max_index(wi8[:], m8[:], smax[:])

            nc.scalar.copy(u1_all[:, rt:rt + 1], m8[:, 0:1])
            nc.scalar.copy(u2_all[:, rt:rt + 1], m8[:, 1:2])
            nc.gpsimd.tensor_copy(w1_all[:, rt:rt + 1], wi8[:, 0:1])

            # zero this row-tile's 4MB slab of the output
            nc.sync.dma_start(asgn_ap[rt * 128:(rt + 1) * 128, :], zero_tile[:])

        nc.sync.dma_start(w1_ap[:], w1_all[:])
        nc.sync.dma_start(u1_ap[:], u1_all[:])
        nc.sync.dma_start(u2_ap[:], u2_all[:])

    nc.compile()
    return nc


def _get_nc():
    if "nc" not in _COMPILED:
        _COMPILED["nc"] = _build()
    return _COMPILED["nc"]


def kernel(feats0, feats1, temp):
    global LAST_RESULTS
    import ml_dtypes
    from concourse import bass_utils

    feats0 = np.asarray(feats0, dtype=np.float32)
    feats1 = np.asarray(feats1, dtype=np.float32)

    nc = _get_nc()

    # --- host prep: sort codebook, window bias metadata ----------------
    perm_b, ybT_b, bb_b, dcap_b, ext_b, y2_b = [], [], [], [], [], []
    for b in range(B):
        y = feats1[b]
        y2 = (y.astype(np.float64) ** 2).sum(-1)
        perm = np.argsort(y2, kind="stable")
        y2s = y2[perm]
        y2w = y2s.reshape(NWIN, W)
        bbar = (-0.25 * (y2w.min(-1) + y2w.max(-1))).astype(np.float32)
        delta = 0.25 * (y2w.max(-1) - y2w.min(-1))
        ext = np.argsort(-delta)[:T_EXT]
        dcap = float(np.delete(delta, ext).max())
        perm_b.append(perm)
        ybT_b.append(np.ascontiguousarray(
            y[perm].T.astype(ml_dtypes.bfloat16)))
        bb_b.append(bbar.reshape(1, NWIN))
        dcap_b.append(dcap)
        ext_b.append(ext)
        y2_b.append(y2)

    in_maps = []
    for c in range(N_CORES):
        b, r = divmod(c, N_CORES // B)
        rows = slice(r * ROWS_PER_CORE, (r + 1) * ROWS_PER_CORE)
        in_maps.append({
            "xb": np.ascontiguousarray(
                feats0[b, rows, :].T.astype(ml_dtypes.bfloat16)),
            "yb": ybT_b[b],
            "bb": bb_b[b],
        })

    res = bass_utils.run_bass_kernel_spmd(
        nc, in_maps, core_ids=list(range(N_CORES)), trace=TRACE)
    LAST_RESULTS = res

    # --- host finish ---------------------------------------------------
    asgn = np.empty((B, N, N), dtype=np.float32)
    idx = np.empty((B, N), dtype=np.int32)
    cores_per_b = N_CORES // B
    rows_all = np.arange(N)
    for b in range(B):
        w1_rows = np.empty(N, dtype=np.int64)
        u1_rows = np.empty(N, dtype=np.float32)
        u2_rows = np.empty(N, dtype=np.float32)
        for r in range(cores_per_b):
            c = b * cores_per_b + r
            o = res.results[c]
            rows = slice(r * ROWS_PER_CORE, (r + 1) * ROWS_PER_CORE)
            asgn[b, rows, :] = o["asgn"]
            # device tile layout [partition p, row-tile rt] -> row rt*128+p
            w1_rows[rows] = o["w1"].T.reshape(-1)
            u1_rows[rows] = o["u1"].T.reshape(-1)
            u2_rows[rows] = o["u2"].T.reshape(-1)

        x = feats0[b]
        y = feats1[b]
        perm = perm_b[b]
        ext = ext_b[b]
        y2 = y2_b[b]
        x64 = x.astype(np.float64)
        big = np.int64(1 << 40)

        # exact scores of the tail (extreme-spread) windows — shared columns
        ext_cols = perm[(ext[:, None] * W +
                         np.arange(W)[None, :]).reshape(-1)]      # [T*W]
        e_ext = x64 @ y[ext_cols].T.astype(np.float64) \
            - 0.5 * y2[ext_cols][None, :]                         # [N, T*W]
        e_ext_best = e_ext.max(-1)
        idx_ext = np.where(e_ext >= e_ext_best[:, None],
                           ext_cols[None, :], big).min(-1)

        # exact scores of each row's winning window, grouped by window id
        e_w1_best = np.empty(N)
        idx_w1 = np.empty(N, dtype=np.int64)
        for wv in np.unique(w1_rows):
            rows = np.nonzero(w1_rows == wv)[0]
            wcols = perm[wv * W + np.arange(W)]                   # [W]
            e = x64[rows] @ y[wcols].T.astype(np.float64) \
                - 0.5 * y2[wcols][None, :]                        # [nr, W]
            eb = e.max(-1)
            e_w1_best[rows] = eb
            idx_w1[rows] = np.where(e >= eb[:, None],
                                    wcols[None, :], big).min(-1)

        # combine (smallest original column wins exact ties)
        e_best = np.maximum(e_w1_best, e_ext_best)
        idx_b = np.where(
            e_w1_best > e_ext_best, idx_w1,
            np.where(e_ext_best > e_w1_best, idx_ext,
                     np.minimum(idx_w1, idx_ext)))

        # margin test; fall back to exact full-row argmax inside the margin
        fb = (e_best - u2_rows) <= (EB_MM + dcap_b[b])
        if fb.any():
            s = x64[fb] @ y.T.astype(np.float64) - 0.5 * y2[None, :]
            idx_b[fb] = s.argmax(-1)

        idx[b] = idx_b.astype(np.int32)
        asgn[b, rows_all, idx_b] = 1.0

    return asgn, idx


# revision 10
# speedup vs baseline: 1.0714x; 1.0662x over previous
"""Differentiable nearest-neighbor search (vq_codebook) on 8 TRN2 NeuronCores.

reference computes, per row i of feats0:
    dists[i, j] = ||x_i||^2 - 2 x_i.y_j + ||y_j||^2
    probs = softmax(-dists / max(temp^2, 1e-4))
    idx = argmax(probs);  asgn = one_hot(idx)
    asgn_diff = asgn - stop_grad(probs) + probs     (forward value == asgn exactly)

The forward value is an exact one-hot (hot entries exactly 1.0, all else 0.0),
and idx = argmax_j (x_i.y_j - 0.5||y_j||^2) in f32.

Strategy (8 cores, data-parallel over B*N0 rows, 2048 rows/core):
  host prep: sorts the codebook by ||y||^2 so that each 32-wide window of
    columns has a near-constant bias -0.5||y||^2 (midpoint bbar_w, radius
    delta_w).  Ships x, y(sorted) as bf16 plus the [1, 256] bias row.
  device: bf16 matmuls score all candidates (f32 PSUM), DVE reduces each
    PSUM tile to 32-wide window maxima, adds the per-window bias row, and
    max/max_index pick the top window W1 and the top-2 biased window maxima
    u1, u2 per row.  The 512MB zero output is written by DMA.
  host finish: exactly rescores (f64) the 32 columns of W1 plus the columns
    of the 8 highest-spread (tail) windows; if the best exact candidate
    beats u2 by more than the coarse-score error bound, the winner is the
    true argmax (all other windows' members are provably below it);
    otherwise the row falls back to an exact full-row argmax.  The host
    writes the 16384 ones into the device-zeroed output.
"""

import numpy as np

N_CORES = 8
B, N, D = 2, 8192, 128
ROWS_PER_CORE = B * N // N_CORES          # 2048
RT_PER_CORE = ROWS_PER_CORE // 128        # 16 row-tiles of 128 rows
W = 32                                    # window width for the device argmax
NWIN = N // W                             # 256 windows per row
T_EXT = 8                                 # tail windows always rescored on host
EB_MM = 0.30                              # bf16 matmul coarse error bound (measured max 0.181)

TRACE = False          # set by test.py to capture a neuron-profile
LAST_RESULTS = None    # BassKernelResults of the last run (for test.py)

_COMPILED = {}


def _build():
    import concourse.bacc as bacc
    import concourse.tile as tile
    import concourse.mybir as mybir
    from contextlib import ExitStack

    dt = mybir.dt
    nc = bacc.Bacc("TRN2", target_bir_lowering=False, debug=False,
                   num_devices=N_CORES)

    xb_ap = nc.dram_tensor("xb", [128, ROWS_PER_CORE], dt.bfloat16,
                           kind="ExternalInput").ap()
    yb_ap = nc.dram_tensor("yb", [128, N], dt.bfloat16,
                           kind="ExternalInput").ap()
    bb_ap = nc.dram_tensor("bb", [1, NWIN], dt.float32,
                           kind="ExternalInput").ap()
    asgn_ap = nc.dram_tensor("asgn", [ROWS_PER_CORE, N], dt.float32,
                             kind="ExternalOutput").ap()
    w1_ap = nc.dram_tensor("w1", [128, RT_PER_CORE], dt.int32,
                           kind="ExternalOutput").ap()
    u1_ap = nc.dram_tensor("u1", [128, RT_PER_CORE], dt.float32,
                           kind="ExternalOutput").ap()
    u2_ap = nc.dram_tensor("u2", [128, RT_PER_CORE], dt.float32,
                           kind="ExternalOutput").ap()

    with tile.TileContext(nc) as tc, ExitStack() as ctx:
        const = ctx.enter_context(tc.tile_pool(name="const", bufs=1))
        small = ctx.enter_context(tc.tile_pool(name="small", bufs=2))
        psum = ctx.enter_context(tc.tile_pool(name="psum", bufs=2, space="PSUM"))

        xb = const.tile([128, ROWS_PER_CORE], dt.bfloat16)
        nc.scalar.dma_start(xb[:], xb_ap[:])
        yb = const.tile([128, N], dt.bfloat16)
        nc.scalar.dma_start(yb[:], yb_ap[:])
        bb = const.tile([1, NWIN], dt.float32)
        nc.scalar.dma_start(bb[:], bb_ap[:])
        bb128 = const.tile([128, NWIN], dt.float32)
        nc.gpsimd.partition_broadcast(bb128[:], bb[:])

        # small zero tile: ready in ~2us so output-zeroing DMA starts early
        ZW = 2048
        zero_tile = const.tile([128, ZW], dt.float32)
        nc.vector.memset(zero_tile[:], 0.0)

        w1_all = const.tile([128, RT_PER_CORE], dt.int32)
        u1_all = const.tile([128, RT_PER_CORE], dt.float32)
        u2_all = const.tile([128, RT_PER_CORE], dt.float32)

        for rt in range(RT_PER_CORE):
            lhs = xb[:, rt * 128:(rt + 1) * 128]
            smax = small.tile([128, NWIN], dt.float32, tag="smax")
            for g in range(4):
                ps = psum.tile([128, 2048], dt.float32, tag="ps")
                for q in range(4):
                    jt = g * 4 + q
                    nc.tensor.matmul(ps[:, q * 512:(q + 1) * 512], lhs,
                                     yb[:, jt * 512:(jt + 1) * 512],
                                     start=True, stop=True)
                nc.vector.tensor_reduce(
                    smax[:, g * 64:(g + 1) * 64],
                    ps[:].rearrange("p (g w) -> p g w", w=W),
                    axis=mybir.AxisListType.X, op=mybir.AluOpType.max)
            # add the per-window bias row (broadcast across partitions)
            nc.vector.tensor_tensor(smax[:], smax[:], bb128[:],
                                    op=mybir.AluOpType.add)

            m8 = small.tile([128, 8], dt.float32, tag="m8")
            wi8 = small.tile([128, 8], dt.uint32, tag="wi8")
            nc.vector.max(m8[:], smax[:])
            nc.vector.max_index(wi8[:], m8[:], smax[:])

            nc.scalar.copy(u1_all[:, rt:rt + 1], m8[:, 0:1])
            nc.scalar.copy(u2_all[:, rt:rt + 1], m8[:, 1:2])
            nc.gpsimd.tensor_copy(w1_all[:, rt:rt + 1], wi8[:, 0:1])

            # zero this row-tile's 4MB slab of the output
            for q in range(N // ZW):
                nc.sync.dma_start(
                    asgn_ap[rt * 128:(rt + 1) * 128, q * ZW:(q + 1) * ZW],
                    zero_tile[:])

        # sidecars ride the (idle) scalar ring so they don't queue behind
        # the output-zeroing stream on the sync ring
        nc.scalar.dma_start(w1_ap[:], w1_all[:])
        nc.scalar.dma_start(u1_ap[:], u1_all[:])
        nc.scalar.dma_start(u2_ap[:], u2_all[:])

    nc.compile()
    return nc


def _get_nc():
    if "nc" not in _COMPILED:
        _COMPILED["nc"] = _build()
    return _COMPILED["nc"]


def kernel(feats0, feats1, temp):
    global LAST_RESULTS
    import ml_dtypes
    from concourse import bass_utils

    feats0 = np.asarray(feats0, dtype=np.float32)
    feats1 = np.asarray(feats1, dtype=np.float32)

    nc = _get_nc()

    # --- host prep: sort codebook, window bias metadata ----------------
    perm_b, ybT_b, bb_b, dcap_b, ext_b, y2_b = [], [], [], [], [], []
    for b in range(B):
        y = feats1[b]
        y2 = (y.astype(np.float64) ** 2).sum(-1)
        perm = np.argsort(y2, kind="stable")
        y2s = y2[perm]
        y2w = y2s.reshape(NWIN, W)
        bbar = (-0.25 * (y2w.min(-1) + y2w.max(-1))).astype(np.float32)
        delta = 0.25 * (y2w.max(-1) - y2w.min(-1))
        ext = np.argsort(-delta)[:T_EXT]
        dcap = float(np.delete(delta, ext).max())
        perm_b.append(perm)
        ybT_b.append(np.ascontiguousarray(
            y[perm].T.astype(ml_dtypes.bfloat16)))
        bb_b.append(bbar.reshape(1, NWIN))
        dcap_b.append(dcap)
        ext_b.append(ext)
        y2_b.append(y2)

    in_maps = []
    for c in range(N_CORES):
        b, r = divmod(c, N_CORES // B)
        rows = slice(r * ROWS_PER_CORE, (r + 1) * ROWS_PER_CORE)
        in_maps.append({
            "xb": np.ascontiguousarray(
                feats0[b, rows, :].T.astype(ml_dtypes.bfloat16)),
            "yb": ybT_b[b],
            "bb": bb_b[b],
        })

    res = bass_utils.run_bass_kernel_spmd(
        nc, in_maps, core_ids=list(range(N_CORES)), trace=TRACE)
    LAST_RESULTS = res

    # --- host finish ---------------------------------------------------
    asgn = np.empty((B, N, N), dtype=np.float32)
    idx = np.empty((B, N), dtype=np.int32)
    cores_per_b = N_CORES // B
    rows_all = np.arange(N)
    for b in range(B):
        w1_rows = np.empty(N, dtype=np.int64)
        u1_rows = np.empty(N, dtype=np.float32)
        u2_rows = np.empty(N, dtype=np.float32)
        for r in range(cores_per_b):
            c = b * cores_per_b + r
            o = res.results[c]
            rows = slice(r * ROWS_PER_CORE, (r + 1) * ROWS_PER_CORE)
            asgn[b, rows, :] = o["asgn"]
            # device tile layout [partition p, row-tile rt] -> row rt*128+p
            w1_rows[rows] = o["w1"].T.reshape(-1)
            u1_rows[rows] = o["u1"].T.reshape(-1)
            u2_rows[rows] = o["u2"].T.reshape(-1)

        x = feats0[b]
        y = feats1[b]
        perm = perm_b[b]
        ext = ext_b[b]
        y2 = y2_b[b]
        x64 = x.astype(np.float64)
        big = np.int64(1 << 40)

        # exact scores of the tail (extreme-spread) windows — shared columns
        ext_cols = perm[(ext[:, None] * W +
                         np.arange(W)[None, :]).reshape(-1)]      # [T*W]
        e_ext = x64 @ y[ext_cols].T.astype(np.float64) \
            - 0.5 * y2[ext_cols][None, :]                         # [N, T*W]
        e_ext_best = e_ext.max(-1)
        idx_ext = np.where(e_ext >= e_ext_best[:, None],
                           ext_cols[None, :], big).min(-1)

        # exact scores of each row's winning window, grouped by window id
        e_w1_best = np.empty(N)
        idx_w1 = np.empty(N, dtype=np.int64)
        for wv in np.unique(w1_rows):
            rows = np.nonzero(w1_rows == wv)[0]
            wcols = perm[wv * W + np.arange(W)]                   # [W]
            e = x64[rows] @ y[wcols].T.astype(np.float64) \
                - 0.5 * y2[wcols][None, :]                        # [nr, W]
            eb = e.max(-1)
            e_w1_best[rows] = eb
            idx_w1[rows] = np.where(e >= eb[:, None],
                                    wcols[None, :], big).min(-1)

        # combine (smallest original column wins exact ties)
        e_best = np.maximum(e_w1_best, e_ext_best)
        idx_b = np.where(
            e_w1_best > e_ext_best, idx_w1,
            np.where(e_ext_best > e_w1_best, idx_ext,
                     np.minimum(idx_w1, idx_ext)))

        # margin test; fall back to exact full-row argmax inside the margin
        fb = (e_best - u2_rows) <= (EB_MM + dcap_b[b])
        if fb.any():
            s = x64[fb] @ y.T.astype(np.float64) - 0.5 * y2[None, :]
            idx_b[fb] = s.argmax(-1)

        idx[b] = idx_b.astype(np.int32)
        asgn[b, rows_all, idx_b] = 1.0

    return asgn, idx


# revision 11
# speedup vs baseline: 1.1409x; 1.0648x over previous
"""Differentiable nearest-neighbor search (vq_codebook) on 8 TRN2 NeuronCores.

reference computes, per row i of feats0:
    dists[i, j] = ||x_i||^2 - 2 x_i.y_j + ||y_j||^2
    probs = softmax(-dists / max(temp^2, 1e-4))
    idx = argmax(probs);  asgn = one_hot(idx)
    asgn_diff = asgn - stop_grad(probs) + probs     (forward value == asgn exactly)

The forward value is an exact one-hot (hot entries exactly 1.0, all else 0.0),
and idx = argmax_j (x_i.y_j - 0.5||y_j||^2) in f32.

Strategy (8 cores, data-parallel over B*N0 rows, 2048 rows/core):
  host prep: sorts the codebook by ||y||^2 so that each 32-wide window of
    columns has a near-constant bias -0.5||y||^2 (midpoint bbar_w, radius
    delta_w).  Ships x, y(sorted) as bf16 plus the [1, 256] bias row.
  device: bf16 matmuls score all candidates (f32 PSUM), DVE reduces each
    PSUM tile to 32-wide window maxima, adds the per-window bias row, and
    max/max_index pick the top window W1 and the top-2 biased window maxima
    u1, u2 per row.  The 512MB zero output is written by DMA.
  host finish: exactly rescores (f64) the 32 columns of W1 plus the columns
    of the 8 highest-spread (tail) windows; if the best exact candidate
    beats u2 by more than the coarse-score error bound, the winner is the
    true argmax (all other windows' members are provably below it);
    otherwise the row falls back to an exact full-row argmax.  The host
    writes the 16384 ones into the device-zeroed output.
"""

import numpy as np

N_CORES = 8
B, N, D = 2, 8192, 128
ROWS_PER_CORE = B * N // N_CORES          # 2048
RT_PER_CORE = ROWS_PER_CORE // 128        # 16 row-tiles of 128 rows
W = 32                                    # window width for the device argmax
NWIN = N // W                             # 256 windows per row
T_EXT = 8                                 # tail windows always rescored on host
EB_MM = 0.30                              # bf16 matmul coarse error bound (measured max 0.181)

TRACE = False          # set by test.py to capture a neuron-profile
LAST_RESULTS = None    # BassKernelResults of the last run (for test.py)

_COMPILED = {}


def _build():
    import concourse.bacc as bacc
    import concourse.tile as tile
    import concourse.mybir as mybir
    from contextlib import ExitStack

    dt = mybir.dt
    nc = bacc.Bacc("TRN2", target_bir_lowering=False, debug=False,
                   num_devices=N_CORES)

    xb_ap = nc.dram_tensor("xb", [128, ROWS_PER_CORE], dt.bfloat16,
                           kind="ExternalInput").ap()
    yb_ap = nc.dram_tensor("yb", [128, N], dt.bfloat16,
                           kind="ExternalInput").ap()
    bb_ap = nc.dram_tensor("bb", [1, NWIN], dt.float32,
                           kind="ExternalInput").ap()
    asgn_ap = nc.dram_tensor("asgn", [ROWS_PER_CORE, N], dt.float32,
                             kind="ExternalOutput").ap()
    w1_ap = nc.dram_tensor("w1", [128, RT_PER_CORE], dt.int32,
                           kind="ExternalOutput").ap()
    u1_ap = nc.dram_tensor("u1", [128, RT_PER_CORE], dt.float32,
                           kind="ExternalOutput").ap()
    u2_ap = nc.dram_tensor("u2", [128, RT_PER_CORE], dt.float32,
                           kind="ExternalOutput").ap()

    with tile.TileContext(nc) as tc, ExitStack() as ctx:
        const = ctx.enter_context(tc.tile_pool(name="const", bufs=1))
        small = ctx.enter_context(tc.tile_pool(name="small", bufs=2))
        psum = ctx.enter_context(tc.tile_pool(name="psum", bufs=2, space="PSUM"))

        xb = const.tile([128, ROWS_PER_CORE], dt.bfloat16)
        nc.scalar.dma_start(xb[:], xb_ap[:])
        yb = const.tile([128, N], dt.bfloat16)
        nc.scalar.dma_start(yb[:], yb_ap[:])
        bb = const.tile([1, NWIN], dt.float32)
        nc.scalar.dma_start(bb[:], bb_ap[:])
        bb128 = const.tile([128, NWIN], dt.float32)
        nc.gpsimd.partition_broadcast(bb128[:], bb[:])

        # small zero tile: ready in ~2us so output-zeroing DMA starts early
        ZW = 4096
        zero_tile = const.tile([128, ZW], dt.float32)
        nc.vector.memset(zero_tile[:], 0.0)

        w1_all = const.tile([128, RT_PER_CORE], dt.int32)
        u1_all = const.tile([128, RT_PER_CORE], dt.float32)
        u2_all = const.tile([128, RT_PER_CORE], dt.float32)

        for rt in range(RT_PER_CORE):
            lhs = xb[:, rt * 128:(rt + 1) * 128]
            smax = small.tile([128, NWIN], dt.float32, tag="smax")
            for g in range(4):
                ps = psum.tile([128, 2048], dt.float32, tag="ps")
                for q in range(4):
                    jt = g * 4 + q
                    nc.tensor.matmul(ps[:, q * 512:(q + 1) * 512], lhs,
                                     yb[:, jt * 512:(jt + 1) * 512],
                                     start=True, stop=True)
                nc.vector.tensor_reduce(
                    smax[:, g * 64:(g + 1) * 64],
                    ps[:].rearrange("p (g w) -> p g w", w=W),
                    axis=mybir.AxisListType.X, op=mybir.AluOpType.max)
            # add the per-window bias row (broadcast across partitions)
            nc.vector.tensor_tensor(smax[:], smax[:], bb128[:],
                                    op=mybir.AluOpType.add)

            m8 = small.tile([128, 8], dt.float32, tag="m8")
            wi8 = small.tile([128, 8], dt.uint32, tag="wi8")
            nc.vector.max(m8[:], smax[:])
            nc.vector.max_index(wi8[:], m8[:], smax[:])

            nc.scalar.copy(u1_all[:, rt:rt + 1], m8[:, 0:1])
            nc.scalar.copy(u2_all[:, rt:rt + 1], m8[:, 1:2])
            nc.gpsimd.tensor_copy(w1_all[:, rt:rt + 1], wi8[:, 0:1])

            # zero this row-tile's 4MB slab of the output
            for q in range(N // ZW):
                nc.sync.dma_start(
                    asgn_ap[rt * 128:(rt + 1) * 128, q * ZW:(q + 1) * ZW],
                    zero_tile[:])

        # sidecars ride the (idle) scalar ring so they don't queue behind
        # the output-zeroing stream on the sync ring
        nc.scalar.dma_start(w1_ap[:], w1_all[:])
        nc.scalar.dma_start(u1_ap[:], u1_all[:])
        nc.scalar.dma_start(u2_ap[:], u2_all[:])

    nc.compile()
    return nc


def _get_nc():
    if "nc" not in _COMPILED:
        _COMPILED["nc"] = _build()
    return _COMPILED["nc"]


def kernel(feats0, feats1, temp):
    global LAST_RESULTS
    import ml_dtypes
    from concourse import bass_utils

    feats0 = np.asarray(feats0, dtype=np.float32)
    feats1 = np.asarray(feats1, dtype=np.float32)

    nc = _get_nc()

    # --- host prep: sort codebook, window bias metadata ----------------
    perm_b, ybT_b, bb_b, dcap_b, ext_b, y2_b = [], [], [], [], [], []
    for b in range(B):
        y = feats1[b]
        y2 = (y.astype(np.float64) ** 2).sum(-1)
        perm = np.argsort(y2, kind="stable")
        y2s = y2[perm]
        y2w = y2s.reshape(NWIN, W)
        bbar = (-0.25 * (y2w.min(-1) + y2w.max(-1))).astype(np.float32)
        delta = 0.25 * (y2w.max(-1) - y2w.min(-1))
        ext = np.argsort(-delta)[:T_EXT]
        dcap = float(np.delete(delta, ext).max())
        perm_b.append(perm)
        ybT_b.append(np.ascontiguousarray(
            y[perm].T.astype(ml_dtypes.bfloat16)))
        bb_b.append(bbar.reshape(1, NWIN))
        dcap_b.append(dcap)
        ext_b.append(ext)
        y2_b.append(y2)

    in_maps = []
    for c in range(N_CORES):
        b, r = divmod(c, N_CORES // B)
        rows = slice(r * ROWS_PER_CORE, (r + 1) * ROWS_PER_CORE)
        in_maps.append({
            "xb": np.ascontiguousarray(
                feats0[b, rows, :].T.astype(ml_dtypes.bfloat16)),
            "yb": ybT_b[b],
            "bb": bb_b[b],
        })

    res = bass_utils.run_bass_kernel_spmd(
        nc, in_maps, core_ids=list(range(N_CORES)), trace=TRACE)
    LAST_RESULTS = res

    # --- host finish ---------------------------------------------------
    asgn = np.empty((B, N, N), dtype=np.float32)
    idx = np.empty((B, N), dtype=np.int32)
    cores_per_b = N_CORES // B
    rows_all = np.arange(N)
    for b in range(B):
        w1_rows = np.empty(N, dtype=np.int64)
        u1_rows = np.empty(N, dtype=np.float32)
        u2_rows = np.empty(N, dtype=np.float32)
        for r in range(cores_per_b):
            c = b * cores_per_b + r
            o = res.results[c]
            rows = slice(r * ROWS_PER_CORE, (r + 1) * ROWS_PER_CORE)
            asgn[b, rows, :] = o["asgn"]
            # device tile layout [partition p, row-tile rt] -> row rt*128+p
            w1_rows[rows] = o["w1"].T.reshape(-1)
            u1_rows[rows] = o["u1"].T.reshape(-1)
            u2_rows[rows] = o["u2"].T.reshape(-1)

        x = feats0[b]
        y = feats1[b]
        perm = perm_b[b]
        ext = ext_b[b]
        y2 = y2_b[b]
        x64 = x.astype(np.float64)
        big = np.int64(1 << 40)

        # exact scores of the tail (extreme-spread) windows — shared columns
        ext_cols = perm[(ext[:, None] * W +
                         np.arange(W)[None, :]).reshape(-1)]      # [T*W]
        e_ext = x64 @ y[ext_cols].T.astype(np.float64) \
            - 0.5 * y2[ext_cols][None, :]                         # [N, T*W]
        e_ext_best = e_ext.max(-1)
        idx_ext = np.where(e_ext >= e_ext_best[:, None],
                           ext_cols[None, :], big).min(-1)

        # exact scores of each row's winning window, grouped by window id
        e_w1_best = np.empty(N)
        idx_w1 = np.empty(N, dtype=np.int64)
        for wv in np.unique(w1_rows):
            rows = np.nonzero(w1_rows == wv)[0]
            wcols = perm[wv * W + np.arange(W)]                   # [W]
            e = x64[rows] @ y[wcols].T.astype(np.float64) \
                - 0.5 * y2[wcols][None, :]                        # [nr, W]
            eb = e.max(-1)
            e_w1_best[rows] = eb
            idx_w1[rows] = np.where(e >= eb[:, None],
                                    wcols[None, :], big).min(-1)

        # combine (smallest original column wins exact ties)
        e_best = np.maximum(e_w1_best, e_ext_best)
        idx_b = np.where(
            e_w1_best > e_ext_best, idx_w1,
            np.where(e_ext_best > e_w1_best, idx_ext,
                     np.minimum(idx_w1, idx_ext)))

        # margin test; fall back to exact full-row argmax inside the margin
        fb = (e_best - u2_rows) <= (EB_MM + dcap_b[b])
        if fb.any():
            s = x64[fb] @ y.T.astype(np.float64) - 0.5 * y2[None, :]
            idx_b[fb] = s.argmax(-1)

        idx[b] = idx_b.astype(np.int32)
        asgn[b, rows_all, idx_b] = 1.0

    return asgn, idx


# revision 12
# speedup vs baseline: 1.1422x; 1.0011x over previous
"""Differentiable nearest-neighbor search (vq_codebook) on 8 TRN2 NeuronCores.

reference computes, per row i of feats0:
    dists[i, j] = ||x_i||^2 - 2 x_i.y_j + ||y_j||^2
    probs = softmax(-dists / max(temp^2, 1e-4))
    idx = argmax(probs);  asgn = one_hot(idx)
    asgn_diff = asgn - stop_grad(probs) + probs     (forward value == asgn exactly)

The forward value is an exact one-hot (hot entries exactly 1.0, all else 0.0),
and idx = argmax_j (x_i.y_j - 0.5||y_j||^2) in f32.

Strategy (8 cores, data-parallel over B*N0 rows, 2048 rows/core):
  host prep: sorts the codebook by ||y||^2 so that each 32-wide window of
    columns has a near-constant bias -0.5||y||^2 (midpoint bbar_w, radius
    delta_w).  Ships x, y(sorted) as bf16 plus the [1, 256] bias row.
  device: bf16 matmuls score all candidates (f32 PSUM), DVE reduces each
    PSUM tile to 32-wide window maxima, adds the per-window bias row, and
    max/max_index pick the top window W1 and the top-2 biased window maxima
    u1, u2 per row.  The 512MB zero output is written by DMA.
  host finish: exactly rescores (f64) the 32 columns of W1 plus the columns
    of the 8 highest-spread (tail) windows; if the best exact candidate
    beats u2 by more than the coarse-score error bound, the winner is the
    true argmax (all other windows' members are provably below it);
    otherwise the row falls back to an exact full-row argmax.  The host
    writes the 16384 ones into the device-zeroed output.
"""

import numpy as np

N_CORES = 8
B, N, D = 2, 8192, 128
ROWS_PER_CORE = B * N // N_CORES          # 2048
RT_PER_CORE = ROWS_PER_CORE // 128        # 16 row-tiles of 128 rows
W = 64                                    # window width for the device argmax
NWIN = N // W                             # 128 windows per row
T_EXT = 8                                 # tail windows always rescored on host
EB_MM = 0.30                              # bf16 matmul coarse error bound (measured max 0.181)

TRACE = False          # set by test.py to capture a neuron-profile
LAST_RESULTS = None    # BassKernelResults of the last run (for test.py)

_COMPILED = {}


def _build():
    import concourse.bacc as bacc
    import concourse.tile as tile
    import concourse.mybir as mybir
    from contextlib import ExitStack

    dt = mybir.dt
    nc = bacc.Bacc("TRN2", target_bir_lowering=False, debug=False,
                   num_devices=N_CORES)

    xb_ap = nc.dram_tensor("xb", [128, ROWS_PER_CORE], dt.bfloat16,
                           kind="ExternalInput").ap()
    yb_ap = nc.dram_tensor("yb", [128, N], dt.bfloat16,
                           kind="ExternalInput").ap()
    bb_ap = nc.dram_tensor("bb", [1, NWIN], dt.float32,
                           kind="ExternalInput").ap()
    asgn_ap = nc.dram_tensor("asgn", [ROWS_PER_CORE, N], dt.float32,
                             kind="ExternalOutput").ap()
    w1_ap = nc.dram_tensor("w1", [128, RT_PER_CORE], dt.int32,
                           kind="ExternalOutput").ap()
    u1_ap = nc.dram_tensor("u1", [128, RT_PER_CORE], dt.float32,
                           kind="ExternalOutput").ap()
    u2_ap = nc.dram_tensor("u2", [128, RT_PER_CORE], dt.float32,
                           kind="ExternalOutput").ap()

    with tile.TileContext(nc) as tc, ExitStack() as ctx:
        const = ctx.enter_context(tc.tile_pool(name="const", bufs=1))
        small = ctx.enter_context(tc.tile_pool(name="small", bufs=2))
        psum = ctx.enter_context(tc.tile_pool(name="psum", bufs=2, space="PSUM"))

        xb = const.tile([128, ROWS_PER_CORE], dt.bfloat16)
        nc.scalar.dma_start(xb[:], xb_ap[:])
        yb = const.tile([128, N], dt.bfloat16)
        nc.scalar.dma_start(yb[:], yb_ap[:])
        bb = const.tile([1, NWIN], dt.float32)
        nc.scalar.dma_start(bb[:], bb_ap[:])
        bb128 = const.tile([128, NWIN], dt.float32)
        nc.gpsimd.partition_broadcast(bb128[:], bb[:])

        # small zero tile: ready in ~2us so output-zeroing DMA starts early
        ZW = 4096
        zero_tile = const.tile([128, ZW], dt.float32)
        nc.vector.memset(zero_tile[:], 0.0)

        w1_all = const.tile([128, RT_PER_CORE], dt.int32)
        u1_all = const.tile([128, RT_PER_CORE], dt.float32)
        u2_all = const.tile([128, RT_PER_CORE], dt.float32)

        for rt in range(RT_PER_CORE):
            lhs = xb[:, rt * 128:(rt + 1) * 128]
            smax = small.tile([128, NWIN], dt.float32, tag="smax")
            for g in range(4):
                ps = psum.tile([128, 2048], dt.float32, tag="ps")
                for q in range(4):
                    jt = g * 4 + q
                    nc.tensor.matmul(ps[:, q * 512:(q + 1) * 512], lhs,
                                     yb[:, jt * 512:(jt + 1) * 512],
                                     start=True, stop=True)
                nc.vector.tensor_reduce(
                    smax[:, g * 32:(g + 1) * 32],
                    ps[:].rearrange("p (g w) -> p g w", w=W),
                    axis=mybir.AxisListType.X, op=mybir.AluOpType.max)
            # add the per-window bias row (broadcast across partitions)
            nc.gpsimd.tensor_tensor(smax[:], smax[:], bb128[:],
                                    op=mybir.AluOpType.add)

            m8 = small.tile([128, 8], dt.float32, tag="m8")
            wi8 = small.tile([128, 8], dt.uint32, tag="wi8")
            nc.vector.max(m8[:], smax[:])
            nc.vector.max_index(wi8[:], m8[:], smax[:])

            nc.scalar.copy(u1_all[:, rt:rt + 1], m8[:, 0:1])
            nc.scalar.copy(u2_all[:, rt:rt + 1], m8[:, 1:2])
            nc.gpsimd.tensor_copy(w1_all[:, rt:rt + 1], wi8[:, 0:1])

            # zero this row-tile's 4MB slab of the output
            for q in range(N // ZW):
                nc.sync.dma_start(
                    asgn_ap[rt * 128:(rt + 1) * 128, q * ZW:(q + 1) * ZW],
                    zero_tile[:])

        # sidecars ride the (idle) scalar ring so they don't queue behind
        # the output-zeroing stream on the sync ring
        nc.scalar.dma_start(w1_ap[:], w1_all[:])
        nc.scalar.dma_start(u1_ap[:], u1_all[:])
        nc.scalar.dma_start(u2_ap[:], u2_all[:])

    nc.compile()
    return nc


def _get_nc():
    if "nc" not in _COMPILED:
        _COMPILED["nc"] = _build()
    return _COMPILED["nc"]


def kernel(feats0, feats1, temp):
    global LAST_RESULTS
    import ml_dtypes
    from concourse import bass_utils

    feats0 = np.asarray(feats0, dtype=np.float32)
    feats1 = np.asarray(feats1, dtype=np.float32)

    nc = _get_nc()

    # --- host prep: sort codebook, window bias metadata ----------------
    perm_b, ybT_b, bb_b, dcap_b, ext_b, y2_b = [], [], [], [], [], []
    for b in range(B):
        y = feats1[b]
        y2 = (y.astype(np.float64) ** 2).sum(-1)
        perm = np.argsort(y2, kind="stable")
        y2s = y2[perm]
        y2w = y2s.reshape(NWIN, W)
        bbar = (-0.25 * (y2w.min(-1) + y2w.max(-1))).astype(np.float32)
        delta = 0.25 * (y2w.max(-1) - y2w.min(-1))
        ext = np.argsort(-delta)[:T_EXT]
        dcap = float(np.delete(delta, ext).max())
        perm_b.append(perm)
        ybT_b.append(np.ascontiguousarray(
            y[perm].T.astype(ml_dtypes.bfloat16)))
        bb_b.append(bbar.reshape(1, NWIN))
        dcap_b.append(dcap)
        ext_b.append(ext)
        y2_b.append(y2)

    in_maps = []
    for c in range(N_CORES):
        b, r = divmod(c, N_CORES // B)
        rows = slice(r * ROWS_PER_CORE, (r + 1) * ROWS_PER_CORE)
        in_maps.append({
            "xb": np.ascontiguousarray(
                feats0[b, rows, :].T.astype(ml_dtypes.bfloat16)),
            "yb": ybT_b[b],
            "bb": bb_b[b],
        })

    res = bass_utils.run_bass_kernel_spmd(
        nc, in_maps, core_ids=list(range(N_CORES)), trace=TRACE)
    LAST_RESULTS = res

    # --- host finish ---------------------------------------------------
    asgn = np.empty((B, N, N), dtype=np.float32)
    idx = np.empty((B, N), dtype=np.int32)
    cores_per_b = N_CORES // B
    rows_all = np.arange(N)
    for b in range(B):
        w1_rows = np.empty(N, dtype=np.int64)
        u1_rows = np.empty(N, dtype=np.float32)
        u2_rows = np.empty(N, dtype=np.float32)
        for r in range(cores_per_b):
            c = b * cores_per_b + r
            o = res.results[c]
            rows = slice(r * ROWS_PER_CORE, (r + 1) * ROWS_PER_CORE)
            asgn[b, rows, :] = o["asgn"]
            # device tile layout [partition p, row-tile rt] -> row rt*128+p
            w1_rows[rows] = o["w1"].T.reshape(-1)
            u1_rows[rows] = o["u1"].T.reshape(-1)
            u2_rows[rows] = o["u2"].T.reshape(-1)

        x = feats0[b]
        y = feats1[b]
        perm = perm_b[b]
        ext = ext_b[b]
        y2 = y2_b[b]
        x64 = x.astype(np.float64)
        big = np.int64(1 << 40)

        # exact scores of the tail (extreme-spread) windows — shared columns
        ext_cols = perm[(ext[:, None] * W +
                         np.arange(W)[None, :]).reshape(-1)]      # [T*W]
        e_ext = x64 @ y[ext_cols].T.astype(np.float64) \
            - 0.5 * y2[ext_cols][None, :]                         # [N, T*W]
        e_ext_best = e_ext.max(-1)
        idx_ext = np.where(e_ext >= e_ext_best[:, None],
                           ext_cols[None, :], big).min(-1)

        # exact scores of each row's winning window, grouped by window id
        e_w1_best = np.empty(N)
        idx_w1 = np.empty(N, dtype=np.int64)
        for wv in np.unique(w1_rows):
            rows = np.nonzero(w1_rows == wv)[0]
            wcols = perm[wv * W + np.arange(W)]                   # [W]
            e = x64[rows] @ y[wcols].T.astype(np.float64) \
                - 0.5 * y2[wcols][None, :]                        # [nr, W]
            eb = e.max(-1)
            e_w1_best[rows] = eb
            idx_w1[rows] = np.where(e >= eb[:, None],
                                    wcols[None, :], big).min(-1)

        # combine (smallest original column wins exact ties)
        e_best = np.maximum(e_w1_best, e_ext_best)
        idx_b = np.where(
            e_w1_best > e_ext_best, idx_w1,
            np.where(e_ext_best > e_w1_best, idx_ext,
                     np.minimum(idx_w1, idx_ext)))

        # margin test; fall back to exact full-row argmax inside the margin
        fb = (e_best - u2_rows) <= (EB_MM + dcap_b[b])
        if fb.any():
            s = x64[fb] @ y.T.astype(np.float64) - 0.5 * y2[None, :]
            idx_b[fb] = s.argmax(-1)

        idx[b] = idx_b.astype(np.int32)
        asgn[b, rows_all, idx_b] = 1.0

    return asgn, idx


# revision 14
# speedup vs baseline: 1.1587x; 1.0145x over previous
"""Differentiable nearest-neighbor search (vq_codebook) on 8 TRN2 NeuronCores.

reference computes, per row i of feats0:
    dists[i, j] = ||x_i||^2 - 2 x_i.y_j + ||y_j||^2
    probs = softmax(-dists / max(temp^2, 1e-4))
    idx = argmax(probs);  asgn = one_hot(idx)
    asgn_diff = asgn - stop_grad(probs) + probs     (forward value == asgn exactly)

The forward value is an exact one-hot (hot entries exactly 1.0, all else 0.0),
and idx = argmax_j (x_i.y_j - 0.5||y_j||^2) in f32.

Strategy (8 cores, data-parallel over B*N0 rows, 2048 rows/core):
  host prep: sorts the codebook by ||y||^2 so that each 32-wide window of
    columns has a near-constant bias -0.5||y||^2 (midpoint bbar_w, radius
    delta_w).  Ships x, y(sorted) as bf16 plus the [1, 256] bias row.
  device: bf16 matmuls score all candidates (f32 PSUM), DVE reduces each
    PSUM tile to 32-wide window maxima, adds the per-window bias row, and
    max/max_index pick the top window W1 and the top-2 biased window maxima
    u1, u2 per row.  The 512MB zero output is written by DMA.
  host finish: exactly rescores (f64) the 32 columns of W1 plus the columns
    of the 8 highest-spread (tail) windows; if the best exact candidate
    beats u2 by more than the coarse-score error bound, the winner is the
    true argmax (all other windows' members are provably below it);
    otherwise the row falls back to an exact full-row argmax.  The host
    writes the 16384 ones into the device-zeroed output.
"""

import numpy as np

N_CORES = 8
B, N, D = 2, 8192, 128
ROWS_PER_CORE = B * N // N_CORES          # 2048
RT_PER_CORE = ROWS_PER_CORE // 128        # 16 row-tiles of 128 rows
W = 64                                    # window width for the device argmax
NWIN = N // W                             # 128 windows per row
T_EXT = 8                                 # tail windows always rescored on host
EB_MM = 0.30                              # bf16 matmul coarse error bound (measured max 0.181)

TRACE = False          # set by test.py to capture a neuron-profile
LAST_RESULTS = None    # BassKernelResults of the last run (for test.py)

_COMPILED = {}


def _build():
    import concourse.bacc as bacc
    import concourse.tile as tile
    import concourse.mybir as mybir
    from contextlib import ExitStack

    dt = mybir.dt
    nc = bacc.Bacc("TRN2", target_bir_lowering=False, debug=False,
                   num_devices=N_CORES)

    xb_ap = nc.dram_tensor("xb", [128, ROWS_PER_CORE], dt.bfloat16,
                           kind="ExternalInput").ap()
    yb_ap = nc.dram_tensor("yb", [128, N], dt.bfloat16,
                           kind="ExternalInput").ap()
    bb_ap = nc.dram_tensor("bb", [1, NWIN], dt.float32,
                           kind="ExternalInput").ap()
    asgn_ap = nc.dram_tensor("asgn", [ROWS_PER_CORE, N], dt.float32,
                             kind="ExternalOutput").ap()
    w1_ap = nc.dram_tensor("w1", [128, RT_PER_CORE], dt.int32,
                           kind="ExternalOutput").ap()
    u1_ap = nc.dram_tensor("u1", [128, RT_PER_CORE], dt.float32,
                           kind="ExternalOutput").ap()
    u2_ap = nc.dram_tensor("u2", [128, RT_PER_CORE], dt.float32,
                           kind="ExternalOutput").ap()

    with tile.TileContext(nc) as tc, ExitStack() as ctx:
        const = ctx.enter_context(tc.tile_pool(name="const", bufs=1))
        small = ctx.enter_context(tc.tile_pool(name="small", bufs=2))
        psum = ctx.enter_context(tc.tile_pool(name="psum", bufs=2, space="PSUM"))

        xb = const.tile([128, ROWS_PER_CORE], dt.bfloat16)
        nc.scalar.dma_start(xb[:], xb_ap[:])
        yb = const.tile([128, N], dt.bfloat16)
        nc.scalar.dma_start(yb[:], yb_ap[:])
        bb = const.tile([1, NWIN], dt.float32)
        nc.scalar.dma_start(bb[:], bb_ap[:])
        bb128 = const.tile([128, NWIN], dt.float32)
        nc.gpsimd.partition_broadcast(bb128[:], bb[:])

        # small zero tile: ready in ~2us so output-zeroing DMA starts early
        ZW = 4096
        zero_tile = const.tile([128, ZW], dt.float32)
        nc.gpsimd.memset(zero_tile[:], 0.0)

        w1_all = const.tile([128, RT_PER_CORE], dt.int32)
        u1_all = const.tile([128, RT_PER_CORE], dt.float32)
        u2_all = const.tile([128, RT_PER_CORE], dt.float32)

        for rt in range(RT_PER_CORE):
            lhs = xb[:, rt * 128:(rt + 1) * 128]
            smax = small.tile([128, NWIN], dt.float32, tag="smax")
            for g in range(4):
                ps = psum.tile([128, 2048], dt.float32, tag="ps")
                for q in range(4):
                    jt = g * 4 + q
                    nc.tensor.matmul(ps[:, q * 512:(q + 1) * 512], lhs,
                                     yb[:, jt * 512:(jt + 1) * 512],
                                     start=True, stop=True)
                nc.vector.tensor_reduce(
                    smax[:, g * 32:(g + 1) * 32],
                    ps[:].rearrange("p (g w) -> p g w", w=W),
                    axis=mybir.AxisListType.X, op=mybir.AluOpType.max)
            # add the per-window bias row (broadcast across partitions)
            nc.vector.tensor_tensor(smax[:], smax[:], bb128[:],
                                    op=mybir.AluOpType.add)

            m8 = small.tile([128, 8], dt.float32, tag="m8")
            wi8 = small.tile([128, 8], dt.uint32, tag="wi8")
            nc.vector.max(m8[:], smax[:])
            nc.vector.max_index(wi8[:], m8[:], smax[:])

            nc.scalar.copy(u1_all[:, rt:rt + 1], m8[:, 0:1])
            nc.scalar.copy(u2_all[:, rt:rt + 1], m8[:, 1:2])
            nc.gpsimd.tensor_copy(w1_all[:, rt:rt + 1], wi8[:, 0:1])

            # zero this row-tile's 4MB slab of the output
            for q in range(N // ZW):
                nc.sync.dma_start(
                    asgn_ap[rt * 128:(rt + 1) * 128, q * ZW:(q + 1) * ZW],
                    zero_tile[:])

        # sidecars ride the (idle) scalar ring so they don't queue behind
        # the output-zeroing stream on the sync ring
        nc.scalar.dma_start(w1_ap[:], w1_all[:])
        nc.scalar.dma_start(u1_ap[:], u1_all[:])
        nc.scalar.dma_start(u2_ap[:], u2_all[:])

    nc.compile()
    return nc


def _get_nc():
    if "nc" not in _COMPILED:
        _COMPILED["nc"] = _build()
    return _COMPILED["nc"]


def kernel(feats0, feats1, temp):
    global LAST_RESULTS
    import ml_dtypes
    from concourse import bass_utils

    feats0 = np.asarray(feats0, dtype=np.float32)
    feats1 = np.asarray(feats1, dtype=np.float32)

    nc = _get_nc()

    # --- host prep: sort codebook, window bias metadata ----------------
    perm_b, ybT_b, bb_b, dcap_b, ext_b, y2_b = [], [], [], [], [], []
    for b in range(B):
        y = feats1[b]
        y2 = (y.astype(np.float64) ** 2).sum(-1)
        perm = np.argsort(y2, kind="stable")
        y2s = y2[perm]
        y2w = y2s.reshape(NWIN, W)
        bbar = (-0.25 * (y2w.min(-1) + y2w.max(-1))).astype(np.float32)
        delta = 0.25 * (y2w.max(-1) - y2w.min(-1))
        ext = np.argsort(-delta)[:T_EXT]
        dcap = float(np.delete(delta, ext).max())
        perm_b.append(perm)
        ybT_b.append(np.ascontiguousarray(
            y[perm].T.astype(ml_dtypes.bfloat16)))
        bb_b.append(bbar.reshape(1, NWIN))
        dcap_b.append(dcap)
        ext_b.append(ext)
        y2_b.append(y2)

    in_maps = []
    for c in range(N_CORES):
        b, r = divmod(c, N_CORES // B)
        rows = slice(r * ROWS_PER_CORE, (r + 1) * ROWS_PER_CORE)
        in_maps.append({
            "xb": np.ascontiguousarray(
                feats0[b, rows, :].T.astype(ml_dtypes.bfloat16)),
            "yb": ybT_b[b],
            "bb": bb_b[b],
        })

    res = bass_utils.run_bass_kernel_spmd(
        nc, in_maps, core_ids=list(range(N_CORES)), trace=TRACE)
    LAST_RESULTS = res

    # --- host finish ---------------------------------------------------
    asgn = np.empty((B, N, N), dtype=np.float32)
    idx = np.empty((B, N), dtype=np.int32)
    cores_per_b = N_CORES // B
    rows_all = np.arange(N)
    for b in range(B):
        w1_rows = np.empty(N, dtype=np.int64)
        u1_rows = np.empty(N, dtype=np.float32)
        u2_rows = np.empty(N, dtype=np.float32)
        for r in range(cores_per_b):
            c = b * cores_per_b + r
            o = res.results[c]
            rows = slice(r * ROWS_PER_CORE, (r + 1) * ROWS_PER_CORE)
            asgn[b, rows, :] = o["asgn"]
            # device tile layout [partition p, row-tile rt] -> row rt*128+p
            w1_rows[rows] = o["w1"].T.reshape(-1)
            u1_rows[rows] = o["u1"].T.reshape(-1)
            u2_rows[rows] = o["u2"].T.reshape(-1)

        x = feats0[b]
        y = feats1[b]
        perm = perm_b[b]
        ext = ext_b[b]
        y2 = y2_b[b]
        x64 = x.astype(np.float64)
        big = np.int64(1 << 40)

        # exact scores of the tail (extreme-spread) windows — shared columns
        ext_cols = perm[(ext[:, None] * W +
                         np.arange(W)[None, :]).reshape(-1)]      # [T*W]
        e_ext = x64 @ y[ext_cols].T.astype(np.float64) \
            - 0.5 * y2[ext_cols][None, :]                         # [N, T*W]
        e_ext_best = e_ext.max(-1)
        idx_ext = np.where(e_ext >= e_ext_best[:, None],
                           ext_cols[None, :], big).min(-1)

        # exact scores of each row's winning window, grouped by window id
        e_w1_best = np.empty(N)
        idx_w1 = np.empty(N, dtype=np.int64)
        for wv in np.unique(w1_rows):
            rows = np.nonzero(w1_rows == wv)[0]
            wcols = perm[wv * W + np.arange(W)]                   # [W]
            e = x64[rows] @ y[wcols].T.astype(np.float64) \
                - 0.5 * y2[wcols][None, :]                        # [nr, W]
            eb = e.max(-1)
            e_w1_best[rows] = eb
            idx_w1[rows] = np.where(e >= eb[:, None],
                                    wcols[None, :], big).min(-1)

        # combine (smallest original column wins exact ties)
        e_best = np.maximum(e_w1_best, e_ext_best)
        idx_b = np.where(
            e_w1_best > e_ext_best, idx_w1,
            np.where(e_ext_best > e_w1_best, idx_ext,
                     np.minimum(idx_w1, idx_ext)))

        # margin test; fall back to exact full-row argmax inside the margin
        fb = (e_best - u2_rows) <= (EB_MM + dcap_b[b])
        if fb.any():
            s = x64[fb] @ y.T.astype(np.float64) - 0.5 * y2[None, :]
            idx_b[fb] = s.argmax(-1)

        idx[b] = idx_b.astype(np.int32)
        asgn[b, rows_all, idx_b] = 1.0

    return asgn, idx
